# revision 13
# baseline (speedup 1.0000x reference)
"""Trainium2 Bass kernel for nn_MixtureOfExpertsModel (moe_routing).

Computes, for x [65536, 1024] and 10 experts with 15 outputs each:
    miu = x @ expert_w^T + expert_b      (per expert)
    xi  = x @ gate_w^T + gate_b          (per expert)
    out = sum_e softmax_e(xi) * miu      -> [65536, 15]

Strategy: pure data parallel over 8 NeuronCores (8192 rows each).
Host-side prep transposes x so the contraction dim (K=1024) lands on SBUF
partitions; both linear maps are fused into one [K, 300] weight so each
128-row tile needs 8 accumulating matmuls (+1 tiny bias matmul via a
ones-row).  Matmuls run in fp16: the PE streams one column per cycle for
16-bit dtypes (vs 2 cycles for fp32r, ~3 for fp32) and HBM traffic halves;
measured end-to-end error vs the fp32 reference is ~3.7e-4 relative
(fp32r: 1.9e-4, bf16: 2e-3).  Softmax over experts needs no max-subtract
(logits are O(1)), so post-processing per row tile is
    exp (Scalar) -> multiply (Vector) -> segmented reduce over experts
    (Vector, expert-contiguous columns) -> batched fast reciprocal +
    final multiply once per supertile (Vector).
"""

import sys

if "/opt/trn_rl_repo" not in sys.path:
    sys.path.insert(0, "/opt/trn_rl_repo")

import numpy as np

import concourse.bass as bass
import concourse.bacc as bacc
import concourse.tile as tile
import concourse.mybir as mybir
from concourse.bass_utils import run_bass_kernel_spmd

F32 = mybir.dt.float32
F32R = mybir.dt.float32r
FP16 = mybir.dt.float16
BF16 = mybir.dt.bfloat16

# compute dtype for the matmuls
MODE = "fp16"            # "fp16" | "f32r"
MDT = FP16 if MODE == "fp16" else F32R
NPDT = np.float16 if MODE == "fp16" else np.float32

BS = 65536
K = 1024
E = 10
O = 15
NCOL = 2 * E * O          # 300: cols 0..149 = expert (o*10+e), 150..299 = gate
NCORES = 8
RPC = BS // NCORES        # rows per core: 8192
KC = K // 128             # 8 contraction chunks
# Variable supertile sizes: a small first tile so real matmuls can start as
# soon as ~1 MB has landed, larger steady-state tiles for DMA efficiency.
SUPS = [256, 1536, 2048, 2048, 1536, 768]
assert sum(SUPS) == RPC
SUP_MAX = max(SUPS)
N_WARMUP = 35             # PE warmup matmuls (HAM clock-gate release)


def _build():
    nc = bacc.Bacc("TRN2", target_bir_lowering=False, debug=False,
                   num_devices=NCORES)
    xt = nc.dram_tensor("xt", [KC, 128, RPC], MDT, kind="ExternalInput").ap()
    wt = nc.dram_tensor("wt", [KC, 128, NCOL], MDT, kind="ExternalInput").ap()
    bias = nc.dram_tensor("bias", [1, NCOL], MDT, kind="ExternalInput").ap()
    ones = nc.dram_tensor("ones", [1, 128], MDT, kind="ExternalInput").ap()
    out = nc.dram_tensor("out", [RPC, O], F32, kind="ExternalOutput").ap()

    with tile.TileContext(nc) as tc:
        with (
            tc.tile_pool(name="const", bufs=1) as cp,
            tc.tile_pool(name="x", bufs=3) as xp,
            tc.tile_pool(name="ps", bufs=6, space="PSUM") as ps_pool,
            tc.tile_pool(name="ep", bufs=6) as ep_pool,
            tc.tile_pool(name="sm", bufs=2) as sm_pool,
            tc.tile_pool(name="ob", bufs=2) as ob_pool,
        ):
            # Warm up the PE's HAM clock gate while the first x supertile is
            # still streaming in: matmuls on a memset tile, no DMA deps.
            wu_in = cp.tile([128, NCOL], BF16, name="wu_in")
            nc.gpsimd.memset(wu_in[:], 0.125)
            wu_ps = ps_pool.tile([128, NCOL], F32, tag="wu", name="wu_ps",
                                 bufs=1)
            for _ in range(N_WARMUP):
                nc.tensor.matmul(
                    wu_ps[:], wu_in[:, 0:128], wu_in[:],
                    start=True, stop=True, skip_group_check=True,
                )

            # Constants ride the Scalar HWDGE ring so they don't queue behind
            # the big x loads on the Sync ring.
            wt_t = cp.tile([128, KC, NCOL], MDT, name="wt_t")
            nc.scalar.dma_start(wt_t[:], wt.rearrange("c p n -> p c n"))
            bias_t = cp.tile([1, NCOL], MDT, name="bias_t")
            nc.scalar.dma_start(bias_t[:], bias[:])
            ones_t = cp.tile([1, 128], MDT, name="ones_t")
            nc.scalar.dma_start(ones_t[:], ones[:])

            row0 = 0
            for t, sup in enumerate(SUPS):
                nsub = sup // 128
                xt_t = xp.tile([128, KC, sup], MDT, tag="xt", name=f"xt_{t}",
                               padded_shape=[128, KC, SUP_MAX])
                nc.sync.dma_start(
                    xt_t[:], xt[:, :, row0:row0 + sup]
                    .rearrange("c p r -> p c r"))
                ob = ob_pool.tile([128, nsub, O], F32, tag="ob", name=f"ob_{t}",
                                  padded_shape=[128, SUP_MAX // 128, O])
                # nd slab: [h(num/den), subtile, o] so the den/num planes are
                # contiguous [128, nsub*15] for the batched recip / final mul.
                ndb = sm_pool.tile([128, 2, nsub, O], F32, tag="ndb",
                                   name=f"ndb_{t}",
                                   padded_shape=[128, 2, SUP_MAX // 128, O])
                for s in range(nsub):
                    psum = ps_pool.tile([128, NCOL], F32, tag="ps",
                                        name=f"ps_{t}_{s}")
                    for c in range(KC):
                        nc.tensor.matmul(
                            psum[:],
                            xt_t[:, c, s * 128:(s + 1) * 128],
                            wt_t[:, c, :],
                            start=(c == 0), stop=False,
                        )
                    nc.tensor.matmul(
                        psum[:], ones_t[:1, :], bias_t[:1, :],
                        start=False, stop=True,
                    )
                    # ep[:, 150:300] = exp(xi); ep[:, 0:150] = exp(xi) * miu
                    ep = ep_pool.tile([128, NCOL], F32, tag="ep",
                                      name=f"ep_{t}_{s}")
                    nc.scalar.activation(
                        ep[:, 150:300], psum[:, 150:300],
                        mybir.ActivationFunctionType.Exp,
                    )
                    nc.vector.tensor_mul(
                        ep[:, 0:150], psum[:, 0:150], ep[:, 150:300])
                    # segmented sum over experts (e contiguous):
                    # ndb[:, 0, s, :] = num, ndb[:, 1, s, :] = den
                    nc.vector.reduce_sum(
                        ndb[:, :, s, :],
                        ep[:].rearrange("p (h o e) -> p h o e", h=2, o=O, e=E),
                        axis=mybir.AxisListType.X,
                    )
                rden = sm_pool.tile([128, nsub, O], F32, tag="rden",
                                    name=f"rden_{t}",
                                    padded_shape=[128, SUP_MAX // 128, O])
                nc.vector.reciprocal_approx_fast(rden[:], ndb[:, 1, :, :])
                nc.vector.tensor_mul(ob[:], ndb[:, 0, :, :], rden[:])
                # rows r = row0 + s*128 + p
                nc.scalar.dma_start(
                    out[row0:row0 + sup, :]
                    .rearrange("(s p) o -> p s o", p=128),
                    ob[:],
                )
                row0 += sup
    nc.compile()
    return nc


_NC = None


def _get_nc():
    global _NC
    if _NC is None:
        _NC = _build()
    return _NC


def _prep_inputs(x, expert_w, expert_b, gate_w, gate_b):
    x = np.ascontiguousarray(np.asarray(x, dtype=np.float32))
    # o-major expert columns (n = o*E + e) so the on-chip segmented reduce
    # over experts reads contiguous runs.
    w = np.concatenate([
        np.asarray(expert_w, np.float32).reshape(E, O, K)
        .transpose(1, 0, 2).reshape(E * O, K),
        np.asarray(gate_w, np.float32).reshape(E, O, K)
        .transpose(1, 0, 2).reshape(E * O, K),
    ], axis=0)                                   # [300, K], col n = o*E + e
    b = np.concatenate([
        np.asarray(expert_b, np.float32).reshape(E, O).T.reshape(E * O),
        np.asarray(gate_b, np.float32).reshape(E, O).T.reshape(E * O),
    ]).reshape(1, NCOL)
    wt = np.ascontiguousarray(
        w.reshape(NCOL, KC, 128).transpose(1, 2, 0).astype(NPDT))
    b = b.astype(NPDT)
    # xt[core, c, p, r] = x[core*RPC + r, c*128 + p]
    xt = np.ascontiguousarray(
        x.reshape(NCORES, RPC, KC, 128).transpose(0, 2, 3, 1).astype(NPDT))
    ones_np = np.ones((1, 128), NPDT)
    in_maps = [{"xt": xt[i], "wt": wt, "bias": b, "ones": ones_np}
               for i in range(NCORES)]
    return in_maps


def _run(in_maps, **kw):
    res = run_bass_kernel_spmd(
        _get_nc(), in_maps, core_ids=list(range(NCORES)), **kw)
    out = np.concatenate([r["out"] for r in res.results], axis=0)
    return out, res


def kernel(x, expert_w, expert_b, gate_w, gate_b):
    in_maps = _prep_inputs(x, expert_w, expert_b, gate_w, gate_b)
    out, _ = _run(in_maps)
    return out


def kernel_traced(x, expert_w, expert_b, gate_w, gate_b, **kw):
    """Like kernel() but returns (out, BassKernelResults) with an NTFF trace."""
    in_maps = _prep_inputs(x, expert_w, expert_b, gate_w, gate_b)
    return _run(in_maps, trace=True, **kw)


# revision 14
# speedup vs baseline: 1.0466x; 1.0466x over previous
"""Trainium2 Bass kernel for nn_MixtureOfExpertsModel (moe_routing).

Computes, for x [65536, 1024] and 10 experts with 15 outputs each:
    miu = x @ expert_w^T + expert_b      (per expert)
    xi  = x @ gate_w^T + gate_b          (per expert)
    out = sum_e softmax_e(xi) * miu      -> [65536, 15]

Strategy: pure data parallel over 8 NeuronCores (8192 rows each).
Host-side prep transposes x so the contraction dim (K=1024) lands on SBUF
partitions; both linear maps are fused into one [K, 300] weight so each
128-row tile needs 8 accumulating matmuls (+1 tiny bias matmul via a
ones-row).  Matmuls run in fp16: the PE streams one column per cycle for
16-bit dtypes (vs 2 cycles for fp32r, ~3 for fp32) and HBM traffic halves;
measured end-to-end error vs the fp32 reference is ~3.7e-4 relative
(fp32r: 1.9e-4, bf16: 2e-3).  Softmax over experts needs no max-subtract
(logits are O(1)), so post-processing per row tile is
    exp (Scalar) -> multiply (Vector) -> segmented reduce over experts
    (Vector, expert-contiguous columns) -> batched fast reciprocal +
    final multiply once per supertile (Vector).
"""

import sys

if "/opt/trn_rl_repo" not in sys.path:
    sys.path.insert(0, "/opt/trn_rl_repo")

import numpy as np

import concourse.bass as bass
import concourse.bacc as bacc
import concourse.tile as tile
import concourse.mybir as mybir
from concourse.bass_utils import run_bass_kernel_spmd

F32 = mybir.dt.float32
F32R = mybir.dt.float32r
FP16 = mybir.dt.float16
BF16 = mybir.dt.bfloat16

# compute dtype for the matmuls
MODE = "fp16"            # "fp16" | "f32r"
MDT = FP16 if MODE == "fp16" else F32R
NPDT = np.float16 if MODE == "fp16" else np.float32

BS = 65536
K = 1024
E = 10
O = 15
NCOL = 2 * E * O          # 300: cols 0..149 = expert (o*10+e), 150..299 = gate
NCORES = 8
RPC = BS // NCORES        # rows per core: 8192
KC = K // 128             # 8 contraction chunks
# Variable supertile sizes: a small first tile so real matmuls can start as
# soon as ~1 MB has landed, larger steady-state tiles for DMA efficiency.
SUPS = [256, 768, 1536, 2048, 2048, 1536]
assert sum(SUPS) == RPC
SUP_MAX = max(SUPS)
N_WARMUP = 35             # PE warmup matmuls (HAM clock-gate release)


def _build():
    nc = bacc.Bacc("TRN2", target_bir_lowering=False, debug=False,
                   num_devices=NCORES)
    xt = nc.dram_tensor("xt", [KC, 128, RPC], MDT, kind="ExternalInput").ap()
    wt = nc.dram_tensor("wt", [KC, 128, NCOL], MDT, kind="ExternalInput").ap()
    bias = nc.dram_tensor("bias", [1, NCOL], MDT, kind="ExternalInput").ap()
    ones = nc.dram_tensor("ones", [1, 128], MDT, kind="ExternalInput").ap()
    out = nc.dram_tensor("out", [RPC, O], F32, kind="ExternalOutput").ap()

    with tile.TileContext(nc) as tc:
        with (
            tc.tile_pool(name="const", bufs=1) as cp,
            tc.tile_pool(name="x", bufs=3) as xp,
            tc.tile_pool(name="ps", bufs=6, space="PSUM") as ps_pool,
            tc.tile_pool(name="ep", bufs=6) as ep_pool,
            tc.tile_pool(name="sm", bufs=2) as sm_pool,
            tc.tile_pool(name="ob", bufs=2) as ob_pool,
        ):
            # Warm up the PE's HAM clock gate while the first x supertile is
            # still streaming in: matmuls on a memset tile, no DMA deps.
            wu_in = cp.tile([128, NCOL], BF16, name="wu_in")
            nc.gpsimd.memset(wu_in[:], 0.125)
            wu_ps = ps_pool.tile([128, NCOL], F32, tag="wu", name="wu_ps",
                                 bufs=1)
            for _ in range(N_WARMUP):
                nc.tensor.matmul(
                    wu_ps[:], wu_in[:, 0:128], wu_in[:],
                    start=True, stop=True, skip_group_check=True,
                )

            # Constants ride the Scalar HWDGE ring so they don't queue behind
            # the big x loads on the Sync ring.
            wt_t = cp.tile([128, KC, NCOL], MDT, name="wt_t")
            nc.scalar.dma_start(wt_t[:], wt.rearrange("c p n -> p c n"))
            bias_t = cp.tile([1, NCOL], MDT, name="bias_t")
            nc.scalar.dma_start(bias_t[:], bias[:])
            ones_t = cp.tile([1, 128], MDT, name="ones_t")
            nc.scalar.dma_start(ones_t[:], ones[:])

            row0 = 0
            for t, sup in enumerate(SUPS):
                nsub = sup // 128
                xt_t = xp.tile([128, KC, sup], MDT, tag="xt", name=f"xt_{t}",
                               padded_shape=[128, KC, SUP_MAX])
                nc.sync.dma_start(
                    xt_t[:], xt[:, :, row0:row0 + sup]
                    .rearrange("c p r -> p c r"))
                ob = ob_pool.tile([128, nsub, O], F32, tag="ob", name=f"ob_{t}",
                                  padded_shape=[128, SUP_MAX // 128, O])
                # nd slab: [h(num/den), subtile, o] so the den/num planes are
                # contiguous [128, nsub*15] for the batched recip / final mul.
                ndb = sm_pool.tile([128, 2, nsub, O], F32, tag="ndb",
                                   name=f"ndb_{t}",
                                   padded_shape=[128, 2, SUP_MAX // 128, O])
                for s in range(nsub):
                    psum = ps_pool.tile([128, NCOL], F32, tag="ps",
                                        name=f"ps_{t}_{s}")
                    for c in range(KC):
                        nc.tensor.matmul(
                            psum[:],
                            xt_t[:, c, s * 128:(s + 1) * 128],
                            wt_t[:, c, :],
                            start=(c == 0), stop=False,
                        )
                    nc.tensor.matmul(
                        psum[:], ones_t[:1, :], bias_t[:1, :],
                        start=False, stop=True,
                    )
                    # ep[:, 150:300] = exp(xi); ep[:, 0:150] = exp(xi) * miu
                    ep = ep_pool.tile([128, NCOL], F32, tag="ep",
                                      name=f"ep_{t}_{s}")
                    nc.scalar.activation(
                        ep[:, 150:300], psum[:, 150:300],
                        mybir.ActivationFunctionType.Exp,
                    )
                    nc.vector.tensor_mul(
                        ep[:, 0:150], psum[:, 0:150], ep[:, 150:300])
                    # segmented sum over experts (e contiguous):
                    # ndb[:, 0, s, :] = num, ndb[:, 1, s, :] = den
                    nc.vector.reduce_sum(
                        ndb[:, :, s, :],
                        ep[:].rearrange("p (h o e) -> p h o e", h=2, o=O, e=E),
                        axis=mybir.AxisListType.X,
                    )
                rden = sm_pool.tile([128, nsub, O], F32, tag="rden",
                                    name=f"rden_{t}",
                                    padded_shape=[128, SUP_MAX // 128, O])
                nc.vector.reciprocal_approx_fast(rden[:], ndb[:, 1, :, :])
                nc.vector.tensor_mul(ob[:], ndb[:, 0, :, :], rden[:])
                # rows r = row0 + s*128 + p
                nc.scalar.dma_start(
                    out[row0:row0 + sup, :]
                    .rearrange("(s p) o -> p s o", p=128),
                    ob[:],
                )
                row0 += sup
    nc.compile()
    return nc


_NC = None


def _get_nc():
    global _NC
    if _NC is None:
        _NC = _build()
    return _NC


def _prep_inputs(x, expert_w, expert_b, gate_w, gate_b):
    x = np.ascontiguousarray(np.asarray(x, dtype=np.float32))
    # o-major expert columns (n = o*E + e) so the on-chip segmented reduce
    # over experts reads contiguous runs.
    w = np.concatenate([
        np.asarray(expert_w, np.float32).reshape(E, O, K)
        .transpose(1, 0, 2).reshape(E * O, K),
        np.asarray(gate_w, np.float32).reshape(E, O, K)
        .transpose(1, 0, 2).reshape(E * O, K),
    ], axis=0)                                   # [300, K], col n = o*E + e
    b = np.concatenate([
        np.asarray(expert_b, np.float32).reshape(E, O).T.reshape(E * O),
        np.asarray(gate_b, np.float32).reshape(E, O).T.reshape(E * O),
    ]).reshape(1, NCOL)
    wt = np.ascontiguousarray(
        w.reshape(NCOL, KC, 128).transpose(1, 2, 0).astype(NPDT))
    b = b.astype(NPDT)
    # xt[core, c, p, r] = x[core*RPC + r, c*128 + p]
    xt = np.ascontiguousarray(
        x.reshape(NCORES, RPC, KC, 128).transpose(0, 2, 3, 1).astype(NPDT))
    ones_np = np.ones((1, 128), NPDT)
    in_maps = [{"xt": xt[i], "wt": wt, "bias": b, "ones": ones_np}
               for i in range(NCORES)]
    return in_maps


def _run(in_maps, **kw):
    res = run_bass_kernel_spmd(
        _get_nc(), in_maps, core_ids=list(range(NCORES)), **kw)
    out = np.concatenate([r["out"] for r in res.results], axis=0)
    return out, res


def kernel(x, expert_w, expert_b, gate_w, gate_b):
    in_maps = _prep_inputs(x, expert_w, expert_b, gate_w, gate_b)
    out, _ = _run(in_maps)
    return out


def kernel_traced(x, expert_w, expert_b, gate_w, gate_b, **kw):
    """Like kernel() but returns (out, BassKernelResults) with an NTFF trace."""
    in_maps = _prep_inputs(x, expert_w, expert_b, gate_w, gate_b)
    return _run(in_maps, trace=True, **kw)
